# revision 28
# baseline (speedup 1.0000x reference)
"""Self-contained BiLSTM-CRF NLL kernel for 8 axon-tunneled TRN2 NeuronCores.

Strategy: data-parallel over the batch (8 sequences/core). kernel(**inputs)
takes the full unsharded inputs, runs the SPMD Bass kernel on cores 0-7, and
combines partial results (gold-path score pieces that involve only integer
tags and the small CRF tables are folded in on the host).

On-chip design (per core, 8 seqs x 256 steps):
 - gather embeddings (indirect DMA), PE-transpose to [d, tok], bf16
 - xg = w_ih.T @ x as bf16 GEMM; only the two boundary chunks (f:0, b:3)
   run up front -- the remaining transposes + GEMM chunks (and the two
   emission chunks that are ready early) are interleaved into the
   recurrence loop to fill Tensor-engine idle windows
 - LSTM recurrence: per step/dir 17 small matmuls (xg preload + fp8 whh);
   one 64-wide sigmoid per dir (g pre-scaled 2x, tanh = 2*sig-1 on DVE);
   emission order is phase-interleaved across the f/b chains and the PE
   group order alternates per step, because the ACT/DVE queues are strict
   FIFO and any cross-chain head-of-line block serializes the chains
 - emissions + exp-domain CRF: forward/backward vector chains cover the
   outer 80 steps each; the middle 2x48 steps are bridged by per-sequence
   TxT transfer-matrix products built concurrently with the chains
"""

import sys, time

sys.path.insert(0, "/opt/trn_rl_repo")

from contextlib import ExitStack

import numpy as np

import concourse.bass as bass
import concourse.tile as tile
from concourse import bacc, mybir



F32 = mybir.dt.float32
BF16 = mybir.dt.bfloat16
FP8 = mybir.dt.float8e4
I32 = mybir.dt.int32

V, D, H, T = 50000, 300, 256, 34
DP = 384  # D padded: 300 data + 1 bias/ones row + zeros
B_LOC = 8
G4 = 4 * H  # 1024
AF = mybir.ActivationFunctionType
ALU = mybir.AluOpType


def build_kernel(S: int) -> bass.Bass:
    NTOK = S * B_LOC
    NTILE = NTOK // 128
    assert NTOK % 128 == 0
    TC = min(512, NTOK)  # token chunk for big GEMMs
    NCHUNK = NTOK // TC
    SH = S // 2

    nc = bacc.Bacc("TRN2", target_bir_lowering=False, debug=False)

    emb = nc.dram_tensor("emb", [V, D], F32, kind="ExternalInput")
    tok_ids = nc.dram_tensor("tok_ids", [128, NTILE], I32, kind="ExternalInput")
    wihT = {
        d: nc.dram_tensor(f"wihT_{d}", [DP, G4], BF16, kind="ExternalInput")
        for d in "fb"
    }
    whhT = {
        d: nc.dram_tensor(f"whhT_{d}", [H, G4], FP8, kind="ExternalInput")
        for d in "fb"
    }
    woutT = nc.dram_tensor("woutT", [2 * H, T], BF16, kind="ExternalInput")
    oh = nc.dram_tensor("oh", [T, NTOK], F32, kind="ExternalInput")
    expT = nc.dram_tensor("expT", [T, T], F32, kind="ExternalInput")
    expTT = nc.dram_tensor("expTT", [T, T], F32, kind="ExternalInput")
    # [T, 1] column vectors
    expStart = nc.dram_tensor("expStart", [T, 1], F32, kind="ExternalInput")
    expEnd = nc.dram_tensor("expEnd", [T, 1], F32, kind="ExternalInput")
    expem_bias = nc.dram_tensor("expem_bias", [T, 1], F32, kind="ExternalInput")
    bout = nc.dram_tensor("bout", [T, 1], F32, kind="ExternalInput")
    ident16 = nc.dram_tensor("ident16", [128, 128], BF16, kind="ExternalInput")
    ident32 = nc.dram_tensor("ident32", [128, 128], F32, kind="ExternalInput")
    out = nc.dram_tensor("out", [1, 2], F32, kind="ExternalOutput")

    with tile.TileContext(nc) as tc, ExitStack() as top:
        cp = top.enter_context(tc.tile_pool(name="const", bufs=1))
        xg_pool = top.enter_context(tc.tile_pool(name="xg", bufs=1))
        hist_pool = top.enter_context(tc.tile_pool(name="hist", bufs=1))

        # ---- constants into SBUF ----
        ids_sb = cp.tile([128, NTILE], I32)
        nc.sync.dma_start(ids_sb[:], tok_ids[:])
        whh_sb = {}
        for d in "fb":
            for k in range(2):
                t_ = cp.tile([128, G4], FP8, tag=f"whh_{d}{k}")
                nc.sync.dma_start(t_[:], whhT[d][128 * k : 128 * (k + 1), :])
                whh_sb[d, k] = t_
        wout_sb = []
        for q in range(4):
            t_ = cp.tile([128, T], BF16, tag=f"wout{q}")
            nc.sync.dma_start(t_[:], woutT[128 * q : 128 * (q + 1), :])
            wout_sb.append(t_)
        i16_sb = cp.tile([128, 128], BF16)
        nc.sync.dma_start(i16_sb[:], ident16[:])
        i32_sb = cp.tile([128, 128], F32)
        nc.sync.dma_start(i32_sb[:], ident32[:])
        expT_sb = cp.tile([T, T], F32, tag="expT")
        nc.sync.dma_start(expT_sb[:], expT[:])
        expTT_sb = cp.tile([T, T], F32, tag="expTT")
        nc.sync.dma_start(expTT_sb[:], expTT[:])
        vec_sb = {}
        for name, dram in (
            ("expStart", expStart),
            ("expEnd", expEnd),
            ("expem_bias", expem_bias),
            ("bout", bout),
        ):
            t_ = cp.tile([T, 1], F32, tag=name)
            nc.sync.dma_start(t_[:], dram[:])
            vec_sb[name] = t_
        ones34 = cp.tile([T, 1], F32, tag="ones34")
        nc.vector.memset(ones34[:], 1.0)
        oh_sb = cp.tile([T, NTOK], F32, tag="oh")
        nc.sync.dma_start(oh_sb[:], oh[:])

        # persistent big tensors
        xg_sb = {d: xg_pool.tile([128, S * 64], BF16, tag=f"xg_{d}", name=f"xg_{d}") for d in "fb"}
        hist_sb = {
            d: hist_pool.tile([128, 2 * NTOK], BF16, tag=f"hist_{d}", name=f"hist_{d}") for d in "fb"
        }
        xg_v = {d: xg_sb[d][:].rearrange("p (s m b) -> p s m b", m=8, b=B_LOC)
                for d in "fb"}

        # ---- pools used by gather/transpose/GEMM (open through recurrence) ----
        mid = ExitStack()
        gp = mid.enter_context(tc.tile_pool(name="gather", bufs=1))
        xtp = mid.enter_context(tc.tile_pool(name="xT", bufs=1))
        wip = mid.enter_context(tc.tile_pool(name="wih", bufs=1))
        ptp = mid.enter_context(tc.tile_pool(name="psum_t", bufs=2, space="PSUM"))
        pxp = mid.enter_context(tc.tile_pool(name="psum_x", bufs=2, space="PSUM"))

        wih_sb = {}
        for d in "fb":
            for k in range(3):
                t_ = wip.tile([128, G4], BF16, tag=f"wih_{d}{k}")
                nc.sync.dma_start(t_[:], wihT[d][128 * k : 128 * (k + 1), :])
                wih_sb[d, k] = t_

        xT = [xtp.tile([128, NTOK], BF16, tag=f"xT{k}", name=f"xT{k}") for k in range(3)]
        # bias/ones row lives at d=320 -> xT[2] partition 64 (32-aligned)
        for p0 in (32, 64, 96):
            nc.vector.memset(xT[2][p0 : p0 + 32, :], 0.0)
        nc.vector.memset(xT[2][64:65, :], 1.0)

        # gather all token-embedding tiles up front (DMA engines, off PE);
        # boundary tiles (chunks f:0 and b:3) first -- they gate the
        # recurrence start.
        GATHER_ORDER = [0, 1, 2, 3, 12, 13, 14, 15, 4, 5, 6, 7, 8, 9, 10, 11]
        x_sb = [None] * NTILE
        for i in GATHER_ORDER:
            t_ = gp.tile([128, D], F32, tag=f"x{i}")
            nc.gpsimd.indirect_dma_start(
                out=t_[:, 0:D],
                out_offset=None,
                in_=emb[:],
                in_offset=bass.IndirectOffsetOnAxis(
                    ap=ids_sb[:, i : i + 1], axis=0
                ),
            )
            x_sb[i] = t_

        # GPSIMD cannot touch PSUM -- alternate PSUM spills between the
        # vector and scalar engines to spread the load.
        _eng = [0]

        def _copy(out, in_):
            _eng[0] ^= 1
            if _eng[0]:
                nc.vector.tensor_copy(out=out, in_=in_)
            else:
                nc.scalar.copy(out=out, in_=in_)

        def transpose_unit(i, k):
            kk = 44 if k == 2 else 128
            pt = ptp.tile([128, 128], F32, tag="pt")
            nc.tensor.transpose(
                out=pt[:kk, :],
                in_=x_sb[i][:, 128 * k : 128 * k + kk],
                identity=i32_sb[:],
            )
            _copy(xT[k][:kk, 128 * i : 128 * (i + 1)], pt[:kk, :])

        def gemm_range(d, m, lo, n):
            px = pxp.tile([128, TC], F32, tag="px", name="px")
            for k in range(3):
                nc.tensor.matmul(
                    px[:, 0:n],
                    lhsT=wih_sb[d, k][:, 128 * m : 128 * (m + 1)],
                    rhs=xT[k][:, lo : lo + n],
                    start=(k == 0),
                    stop=(k == 2),
                )
            s_lo, ns = lo // B_LOC, n // B_LOC
            hc = max(ns // 2, 1)
            pxv = px[:].rearrange("p (s b) -> p s b", b=B_LOC)
            _copy(xg_v[d][:, s_lo : s_lo + hc, m, :], pxv[:, 0:hc, :])
            if hc < ns:
                _copy(xg_v[d][:, s_lo + hc : s_lo + ns, m, :], pxv[:, hc:ns, :])

        def gemm_unit(d, j, m):
            gemm_range(d, m, TC * j, TC)

        # up-front: boundary chunks so the recurrence can start
        for i in (0, 1, 2, 3, 12, 13, 14, 15):
            for k in range(3):
                transpose_unit(i, k)
        for m in range(8):
            gemm_unit("f", 0, m)
        for m in range(8):
            gemm_unit("b", 3, m)

        # interleaved work units, emitted inside the recurrence loop
        foreign: dict[int, list] = {}

        def sched(t, fn, *a):
            foreign.setdefault(t, []).append((fn, a))

        # emissions: exp(emit) for the CRF and raw emit for the gold score.
        # Chunks 1/2 have both hf and hb ready at t=192 and run inside the
        # recurrence; chunks 0/3 only complete at the very end.
        expem_sb = cp.tile([T, NTOK], F32, tag="expem")
        emit_sb = cp.tile([T, NTOK], F32, tag="emit")
        rhs4 = [
            hist_sb["f"][:, 0:NTOK],
            hist_sb["f"][:, NTOK : 2 * NTOK],
            hist_sb["b"][:, 0:NTOK],
            hist_sb["b"][:, NTOK : 2 * NTOK],
        ]

        def emission_unit(j, pool):
            pe_full = pool.tile([128, TC], F32, tag="px", name="pe_full")
            pe_ = pe_full[0:T, :]
            for q in range(4):
                nc.tensor.matmul(
                    pe_,
                    lhsT=wout_sb[q][:],
                    rhs=rhs4[q][:, TC * j : TC * (j + 1)],
                    start=(q == 0),
                    stop=(q == 3),
                )
            nc.scalar.activation(
                expem_sb[:, TC * j : TC * (j + 1)], pe_, AF.Exp,
                bias=vec_sb["expem_bias"][:, 0:1],
            )
            nc.scalar.activation(
                emit_sb[:, TC * j : TC * (j + 1)], pe_, AF.Identity,
                bias=vec_sb["bout"][:, 0:1],
            )

        slot = 2  # first recurrence step that takes foreign work
        for i in (4, 5, 6, 7):
            for k in range(3):
                sched(slot, transpose_unit, i, k); slot += 1
        for m in range(8):
            sched(slot, gemm_unit, "f", 1, m); slot += 1
        for i in (8, 9, 10, 11):
            for k in range(3):
                sched(slot, transpose_unit, i, k); slot += 1
        for m in range(8):
            sched(slot, gemm_unit, "b", 2, m); slot += 1
        assert slot < 62, slot
        slot = 66
        for m in range(8):
            sched(slot, gemm_unit, "f", 2, m); slot += 2
        for m in range(8):
            sched(slot, gemm_unit, "b", 1, m); slot += 2
        assert slot < 126, slot
        slot = 130
        for m in range(8):
            sched(slot, gemm_unit, "f", 3, m); slot += 2
        for m in range(8):
            sched(slot, gemm_unit, "b", 0, m); slot += 2
        assert slot < 190, slot
        sched(194, emission_unit, 1, pxp)
        sched(197, emission_unit, 2, pxp)

        # ---- recurrence ----
        with ExitStack() as ph:
            pgp = {
                d: ph.enter_context(
                    tc.tile_pool(name=f"psum_g{d}", bufs=2, space="PSUM")
                )
                for d in "fb"
            }
            sp = ph.enter_context(tc.tile_pool(name="gates", bufs=4))
            cpool = ph.enter_context(tc.tile_pool(name="cstate", bufs=2))
            hist_v = {
                d: hist_sb[d][:].rearrange("p (k s b) -> p k s b", k=2, b=B_LOC)
                for d in "fb"
            }
            c_prev = {d: None for d in "fb"}
            # phase-interleaved emission across the f/b chains: the ACT and
            # DVE queues are strict FIFO, so a per-direction emission order
            # head-of-line-blocks one chain behind the other's tail ops.
            for t in range(S):
                dirs = [("f", t, t - 1), ("b", S - 1 - t, S - t)]
                if t % 2:  # balance chain latencies: alternate PE group order
                    dirs.reverse()
                pg, sg, thg, t2, t3, cn, thc = {}, {}, {}, {}, {}, {}, {}
                for d, s, s_prev in dirs:
                    pg[d] = pgp[d].tile([128, 64], F32, tag=f"pg{d}", name=f"pg_{d}")
                    nc.tensor.matmul(
                        pg[d][:],
                        lhsT=i16_sb[:],
                        rhs=xg_sb[d][:, 64 * s : 64 * (s + 1)],
                        start=True,
                        stop=(t == 0),
                        skip_group_check=True,
                    )
                    if t > 0:
                        for m in range(8):
                            for k in range(2):
                                nc.tensor.matmul(
                                    pg[d][:, 8 * m : 8 * (m + 1)],
                                    lhsT=whh_sb[d, k][:, 128 * m : 128 * (m + 1)],
                                    rhs=hist_v[d][:, k, s_prev, :],
                                    start=False,
                                    stop=(k == 1 and m == 7),
                                    skip_group_check=True,
                                )
                # per-chain act pairs back-to-back on the ACT FIFO (no
                # inter-sem): sigmoid(i,f,o) then tanh(g) -- the g columns
                # hold 2g (pre-scaled), so Tanh with scale=0.5 gives tanh(g)
                # and the DVE 2s-1 fixup hop disappears from the chain.
                for d, s, s_prev in dirs:
                    sg[d] = sp.tile([128, 48], F32, tag=f"sig{d}", name=f"sg_{d}")
                    thg[d] = sp.tile([128, 16], F32, tag=f"tg{d}", name=f"thg_{d}")
                    nc.scalar.activation(sg[d][:], pg[d][:, 0:48], AF.Sigmoid)
                    nc.scalar.activation(thg[d][:], pg[d][:, 48:64], AF.Tanh,
                                         scale=0.5)
                # DVE ops in expected execution-time order: the queue is
                # strict FIFO, so the lead chain's tail must not sit behind
                # the lag chain's head ops.
                for d, s, s_prev in dirs:
                    t2[d] = sp.tile([128, 16], F32, tag=f"t2{d}", name=f"t2_{d}")
                    t3[d] = sp.tile([128, 16], F32, tag=f"t3{d}", name=f"t3_{d}")
                    cn[d] = cpool.tile([128, 16], F32, tag=f"c{d}", name=f"cn_{d}")
                    if t == 0:
                        nc.vector.tensor_tensor(
                            out=cn[d][:], in0=sg[d][:, 0:16], in1=thg[d][:],
                            op=ALU.mult,
                        )
                    else:
                        nc.vector.tensor_tensor(
                            out=t3[d][:], in0=sg[d][:, 16:32], in1=c_prev[d][:],
                            op=ALU.mult,
                        )
                        nc.vector.tensor_tensor(
                            out=t2[d][:], in0=sg[d][:, 0:16], in1=thg[d][:],
                            op=ALU.mult,
                        )
                        nc.vector.tensor_add(out=cn[d][:], in0=t2[d][:], in1=t3[d][:])
                for d, s, s_prev in dirs:
                    c_prev[d] = cn[d]
                    thc[d] = sp.tile([128, 16], F32, tag=f"thc{d}", name=f"thc_{d}")
                    nc.scalar.activation(thc[d][:], cn[d][:], AF.Tanh)
                for d, s, s_prev in dirs:
                    nc.vector.tensor_tensor(
                        out=hist_v[d][:, :, s, :],
                        in0=sg[d][:, 32:48].rearrange("p (k b) -> p k b", b=B_LOC),
                        in1=thc[d][:].rearrange("p (k b) -> p k b", b=B_LOC),
                        op=ALU.mult,
                    )
                for fn, a in foreign.get(t, ()):
                    fn(*a)

        mid.close()  # free gather/xT/wih SBUF + transpose/GEMM PSUM banks

        # ---- remaining emissions + gold emission score ----
        acc_em = cp.tile([T, 1], F32, tag="acc_em")
        ps_out = top.enter_context(tc.tile_pool(name="psum_o", bufs=1, space="PSUM"))
        po_num = ps_out.tile([1, 1], F32, tag="po_num")
        po_den = ps_out.tile([1, B_LOC], F32, tag="po_den")
        with ExitStack() as ph:
            pep = ph.enter_context(tc.tile_pool(name="psum_e", bufs=2, space="PSUM"))
            ep = ph.enter_context(tc.tile_pool(name="emitp", bufs=1))
            prod_sb = ep.tile([T, NTOK], F32, tag="prod")
            for j in (0, 3):
                emission_unit(j, pep)
            nc.vector.scalar_tensor_tensor(
                out=prod_sb[:],
                in0=emit_sb[:],
                scalar=0.0,
                in1=oh_sb[:],
                op0=ALU.add,
                op1=ALU.mult,
                accum_out=acc_em[:],
            )
            nc.tensor.matmul(
                po_num[:], lhsT=ones34[:], rhs=acc_em[:], start=True, stop=True
            )

        # ---- CRF exp-domain chains, 4-way segmented ----
        # Vector chains cover s in [0, VLEN) fwd and (S-VLEN, S-1] bwd; the
        # middle is bridged by two per-sequence TxT transfer-matrix products
        # G1 (s in [VLEN, VLEN+ML)) and G2 (s in [VLEN+ML, S-VLEN)), built in
        # parallel with the vector chains, then applied transposed:
        # den = Ea_{V-1} . (G1^T G2^T E Eb_{S-V}).
        VLEN = 80
        ML = (S - 2 * VLEN) // 2
        assert S == 2 * VLEN + 2 * ML
        with ExitStack() as ph:
            crf = ph.enter_context(tc.tile_pool(name="crf", bufs=3))
            pcp = ph.enter_context(tc.tile_pool(name="psum_c", bufs=2, space="PSUM"))
            pmp = {
                g: ph.enter_context(
                    tc.tile_pool(name=f"psum_m{g}", bufs=2, space="PSUM"))
                for g in (1, 2)
            }
            gmp = ph.enter_context(tc.tile_pool(name="gmats", bufs=2))

            em = lambda s: expem_sb[:, B_LOC * s : B_LOC * (s + 1)]
            emv = expem_sb[:].rearrange("p (s b) -> p s b", b=B_LOC)
            # paired chains: cols 0:8 = Ea (fwd), cols 8:16 = Eb (bwd)
            eab = crf.tile([T, 2 * B_LOC], F32, tag="eab")
            nc.vector.tensor_scalar_mul(eab[:, 0:B_LOC], em(0), vec_sb["expStart"][:, 0:1])
            nc.vector.tensor_scalar_mul(eab[:, B_LOC:], em(S - 1), vec_sb["expEnd"][:, 0:1])

            # bf16 E for the G-chain matmuls (fp32 lhsT can't pair w/ bf16 rhs)
            expT16 = crf.tile([T, T], BF16, tag="expT16")
            nc.vector.tensor_copy(out=expT16[:], in_=expT_sb[:])
            # G init: diag(em_s0) E^T == row-scaled expTT (bf16, per seq)
            G = {}
            for g, s0 in ((1, VLEN), (2, VLEN + ML)):
                Gt = gmp.tile([T, B_LOC * T], BF16, tag=f"G{g}", name=f"G{g}")
                for b in range(B_LOC):
                    nc.vector.tensor_scalar_mul(
                        Gt[:, T * b : T * (b + 1)], expTT_sb[:],
                        emv[:, s0, b : b + 1])
                G[g] = Gt

            for r in range(1, VLEN):
                # fwd: Ea_r = em(r) * expT.T @ Ea_{r-1}
                # bwd: Eb_{S-1-r} = em(S-1-r) * (Eb chain);  em pair strided AP
                pcab = pcp.tile([T, 2 * B_LOC], F32, tag="pcab")
                nc.tensor.matmul(pcab[:, 0:B_LOC], lhsT=expT_sb[:], rhs=eab[:, 0:B_LOC],
                                 start=True, stop=True)
                nc.tensor.matmul(pcab[:, B_LOC:], lhsT=expTT_sb[:], rhs=eab[:, B_LOC:],
                                 start=True, stop=True)
                eab = crf.tile([T, 2 * B_LOC], F32, tag="eab")
                em_pair = emv[:, r : S - r : S - 1 - 2 * r, :]
                nc.vector.tensor_tensor(
                    out=eab[:].rearrange("p (c b) -> p c b", b=B_LOC),
                    in0=pcab[:].rearrange("p (c b) -> p c b", b=B_LOC),
                    in1=em_pair,
                    op=ALU.mult,
                )
                if r < ML:
                    for g, s0 in ((1, VLEN), (2, VLEN + ML)):
                        pm = pmp[g].tile([T, B_LOC * T], F32, tag=f"pm{g}",
                                         name=f"pm_{g}")
                        nc.tensor.matmul(pm[:], lhsT=expT16[:], rhs=G[g][:],
                                         start=True, stop=True)
                        Gn = gmp.tile([T, B_LOC * T], BF16, tag=f"G{g}",
                                      name=f"Gn_{g}")
                        nc.vector.tensor_tensor(
                            out=Gn[:].rearrange("p (b i) -> p b i", b=B_LOC),
                            in0=pm[:].rearrange("p (b i) -> p b i", b=B_LOC),
                            in1=emv[:, s0 + r, :].to_broadcast([T, B_LOC, T]),
                            op=ALU.mult,
                        )
                        G[g] = Gn

            # tail: v1 = E Eb_{S-V}; v2 = G2^T v1; v3 = G1^T v2;
            # den8 = ones^T (Ea_{V-1} * v3)
            pv1 = pcp.tile([T, B_LOC], F32, tag="pcab")
            nc.tensor.matmul(pv1[:], lhsT=expTT_sb[:], rhs=eab[:, B_LOC:],
                             start=True, stop=True)
            v1 = crf.tile([T, B_LOC], BF16, tag="v1")
            nc.vector.tensor_copy(out=v1[:], in_=pv1[:])
            pv2 = pcp.tile([T, B_LOC], F32, tag="pcab")
            for b in range(B_LOC):
                nc.tensor.matmul(pv2[:, b : b + 1], lhsT=G[2][:, T * b : T * (b + 1)],
                                 rhs=v1[:, b : b + 1], start=True, stop=True)
            v2 = crf.tile([T, B_LOC], BF16, tag="v1")
            nc.vector.tensor_copy(out=v2[:], in_=pv2[:])
            pv3 = pcp.tile([T, B_LOC], F32, tag="pcab")
            for b in range(B_LOC):
                nc.tensor.matmul(pv3[:, b : b + 1], lhsT=G[1][:, T * b : T * (b + 1)],
                                 rhs=v2[:, b : b + 1], start=True, stop=True)
            z = crf.tile([T, B_LOC], F32, tag="z")
            nc.vector.tensor_tensor(out=z[:], in0=eab[:, 0:B_LOC], in1=pv3[:], op=ALU.mult)
            nc.tensor.matmul(po_den[:], lhsT=ones34[:], rhs=z[:], start=True, stop=True)

            outv = crf.tile([1, 2], F32, tag="outv")
            den8 = crf.tile([1, B_LOC], F32, tag="den8")
            nc.scalar.activation(den8[:], po_den[:], AF.Ln)
            nc.vector.reduce_sum(
                out=outv[:, 1:2], in_=den8[:], axis=mybir.AxisListType.X
            )
            nc.vector.tensor_copy(out=outv[:, 0:1], in_=po_num[:])
            nc.sync.dma_start(out[:], outv[:])

    nc.compile()
    return nc


# ----- host-side preprocessing -----
GATE_PERM = np.concatenate(
    [np.arange(0, 2 * H), np.arange(3 * H, 4 * H), np.arange(2 * H, 3 * H)]
)


def prep_shared(w_ih_f, w_hh_f, b_f, w_ih_b, w_hh_b, b_b, w_out, b_out,
                start_t, end_t, trans):
    """Per-core-replicated tensors, keyed by dram tensor name."""
    import ml_dtypes
    out = {}
    for d, w_ih, b in (("f", w_ih_f, b_f), ("b", w_ih_b, b_b)):
        wp = np.zeros((DP, G4), np.float32)
        wp[:D] = w_ih[GATE_PERM].T.astype(np.float32)
        wp[320] = b[GATE_PERM].astype(np.float32)  # bias row at 32-aligned partition
        wp[:, 3 * H :] *= 2.0  # g-gate pre-scale: tanh(x) = 2*sigmoid(2x) - 1
        out[f"wihT_{d}"] = wp.astype(ml_dtypes.bfloat16)
    for d, w_hh in (("f", w_hh_f), ("b", w_hh_b)):
        whp = w_hh[GATE_PERM].T.astype(np.float32)
        whp[:, 3 * H :] *= 2.0  # g-gate pre-scale for the sigmoid-only gate op
        out[f"whhT_{d}"] = whp.astype(ml_dtypes.float8_e4m3)
    out["woutT"] = w_out.T.astype(ml_dtypes.bfloat16)
    out["expT"] = np.exp(trans).astype(np.float32)
    out["expTT"] = np.exp(trans).T.copy().astype(np.float32)
    out["expStart"] = np.exp(start_t).astype(np.float32).reshape(T, 1)
    out["expEnd"] = np.exp(end_t).astype(np.float32).reshape(T, 1)
    out["expem_bias"] = (b_out - np.log(T)).astype(np.float32).reshape(T, 1)
    out["bout"] = b_out.astype(np.float32).reshape(T, 1)
    out["ident16"] = np.eye(128, dtype=ml_dtypes.bfloat16)
    out["ident32"] = np.eye(128, dtype=np.float32)
    return out


def prep_core(batch_sh, tags_sh, S):
    """Per-core tensors from this core's [B_LOC, S] int shards."""
    ntok = S * B_LOC
    ntile = ntok // 128
    ids_flat = batch_sh.T.reshape(-1).astype(np.int32)  # s-major token order
    tok_ids = ids_flat.reshape(ntile, 128).T.copy()
    oh = np.zeros((T, ntok), np.float32)
    tags_flat = tags_sh.T.reshape(-1)  # [ntok] s-major
    oh[tags_flat, np.arange(ntok)] = 1.0
    return {"tok_ids": tok_ids, "oh": oh}


def num_host(tags, start_t, end_t, trans):
    """Tag-path score pieces that don't involve emissions. tags: [B, S]."""
    return float(
        start_t[tags[:, 0]].sum()
        + trans[tags[:, :-1], tags[:, 1:]].sum()
        + end_t[tags[:, -1]].sum()
    )



# ---------------------------------------------------------------------------
# SPMD runner (the run_bass_kernel_spmd axon path, kept open for re-timing).

S_FULL = 256
N_CORES = 8
LAST_EXEC_NS = None
LAST_IN_MAPS = None

_built = {}


def _get_nc():
    if "nc" not in _built:
        _built["nc"] = build_kernel(S_FULL)
    return _built["nc"]


def _run_spmd_timed(nc, in_maps, n_reps=3):
    """bass2jax.run_bass_via_pjrt equivalent that keeps the jitted executable
    and device-resident inputs so pure-execution time can be measured."""
    global LAST_EXEC_NS
    import jax
    from jax.sharding import Mesh, PartitionSpec, NamedSharding
    from jax.experimental.shard_map import shard_map
    from concourse import bass2jax
    from concourse.bass2jax import _bass_exec_p, partition_id_tensor

    bass2jax.install_neuronx_cc_hook()
    partition_name = nc.partition_id_tensor.name if nc.partition_id_tensor else None

    in_names, out_names, out_avals, zero_outs = [], [], [], []
    for alloc in nc.m.functions[0].allocations:
        if not isinstance(alloc, mybir.MemoryLocationSet):
            continue
        name = alloc.memorylocations[0].name
        if alloc.kind == "ExternalInput":
            if name != partition_name:
                in_names.append(name)
        elif alloc.kind == "ExternalOutput":
            shape = tuple(alloc.tensor_shape)
            dtype = mybir.dt.np(alloc.dtype)
            out_names.append(name)
            out_avals.append(jax.core.ShapedArray(shape, dtype))
            zero_outs.append(np.zeros(shape, dtype))
    n_params = len(in_names)
    n_outs = len(out_avals)
    in_names.extend(out_names)
    if partition_name is not None:
        in_names.append(partition_name)

    donate = tuple(range(n_params, n_params + n_outs))

    def _body(*args):
        operands = list(args)
        if partition_name is not None:
            operands.append(partition_id_tensor())
        return tuple(
            _bass_exec_p.bind(
                *operands,
                out_avals=tuple(out_avals),
                in_names=tuple(in_names),
                out_names=tuple(out_names),
                lowering_input_output_aliases=(),
                sim_require_finite=True,
                sim_require_nnan=True,
                nc=nc,
            )
        )

    devices = jax.devices()[:N_CORES]
    mesh = Mesh(np.asarray(devices), ("core",))
    in_specs = (PartitionSpec("core"),) * (n_params + n_outs)
    out_specs = (PartitionSpec("core"),) * n_outs
    sharded = jax.jit(
        shard_map(_body, mesh=mesh, in_specs=in_specs, out_specs=out_specs,
                  check_rep=False),
        donate_argnums=donate,
        keep_unused=True,
    )
    sh = NamedSharding(mesh, PartitionSpec("core"))
    concat_in = [
        jax.device_put(
            np.concatenate([np.asarray(m[in_names[i]]) for m in in_maps], axis=0), sh
        )
        for i in range(n_params)
    ]

    def zeros():
        return [np.zeros((N_CORES * z.shape[0], *z.shape[1:]), z.dtype)
                for z in zero_outs]

    out_arrs = [np.asarray(a) for a in sharded(*concat_in, *zeros())]
    times = []
    for _ in range(n_reps):
        t0 = time.perf_counter()
        r = sharded(*concat_in, *zeros())
        jax.block_until_ready(r)
        times.append(time.perf_counter() - t0)
    if times:
        LAST_EXEC_NS = int(min(times) * 1e9)
    return [
        {name: out_arrs[i].reshape(N_CORES, *out_avals[i].shape)[c]
         for i, name in enumerate(out_names)}
        for c in range(N_CORES)
    ]


def kernel(batch, tags, seq_lengths, emb, w_ih_f, w_hh_f, b_f,
           w_ih_b, w_hh_b, b_b, w_out, b_out, start_t, end_t, trans):
    global LAST_IN_MAPS
    batch = np.asarray(batch)
    tags = np.asarray(tags)
    emb = np.asarray(emb, np.float32)
    w_out_ = np.asarray(w_out, np.float32)
    b_out_ = np.asarray(b_out, np.float32)
    start_t = np.asarray(start_t, np.float32)
    end_t = np.asarray(end_t, np.float32)
    trans = np.asarray(trans, np.float32)
    S = batch.shape[1]
    assert S == S_FULL and batch.shape[0] == N_CORES * B_LOC

    shared = prep_shared(np.asarray(w_ih_f, np.float32), np.asarray(w_hh_f, np.float32),
                         np.asarray(b_f, np.float32), np.asarray(w_ih_b, np.float32),
                         np.asarray(w_hh_b, np.float32), np.asarray(b_b, np.float32),
                         w_out_, b_out_, start_t, end_t, trans)
    shared["emb"] = emb
    in_maps = []
    for c in range(N_CORES):
        m = dict(shared)
        m.update(prep_core(batch[B_LOC * c : B_LOC * (c + 1)].astype(np.int64),
                           tags[B_LOC * c : B_LOC * (c + 1)].astype(np.int64), S))
        in_maps.append(m)
    LAST_IN_MAPS = in_maps

    nc = _get_nc()
    res = _run_spmd_timed(nc, in_maps)

    num_em_tot = 0.0
    den_raw_tot = 0.0
    for c in range(N_CORES):
        o = np.asarray(res[c]["out"], np.float64).reshape(2)
        num_em_tot += o[0]
        den_raw_tot += o[1]
    den_true_tot = den_raw_tot + N_CORES * B_LOC * S * np.log(T)
    nh = num_host(tags, start_t.astype(np.float64), end_t.astype(np.float64),
                  trans.astype(np.float64))
    llh_tot = nh + num_em_tot - den_true_tot
    loss = -llh_tot / (N_CORES * B_LOC)
    return np.asarray(loss, dtype=np.float32)


# revision 29
# speedup vs baseline: 1.1514x; 1.1514x over previous
"""Self-contained BiLSTM-CRF NLL kernel for 8 axon-tunneled TRN2 NeuronCores.

Strategy: data-parallel over the batch (8 sequences/core). kernel(**inputs)
takes the full unsharded inputs, runs the SPMD Bass kernel on cores 0-7, and
combines partial results (gold-path score pieces that involve only integer
tags and the small CRF tables are folded in on the host).

On-chip design (per core, 8 seqs x 256 steps):
 - gather embeddings (indirect DMA), PE-transpose to [d, tok], bf16
 - xg = w_ih.T @ x as bf16 GEMM; only the two boundary chunks (f:0, b:3)
   run up front -- the remaining transposes + GEMM chunks (and the two
   emission chunks that are ready early) are interleaved into the
   recurrence loop to fill Tensor-engine idle windows
 - LSTM recurrence: per step/dir 17 small matmuls (xg preload + fp8 whh);
   one 64-wide sigmoid per dir (g pre-scaled 2x, tanh = 2*sig-1 on DVE);
   emission order is phase-interleaved across the f/b chains and the PE
   group order alternates per step, because the ACT/DVE queues are strict
   FIFO and any cross-chain head-of-line block serializes the chains
 - emissions + exp-domain CRF: forward/backward vector chains cover the
   outer 80 steps each; the middle 2x48 steps are bridged by per-sequence
   TxT transfer-matrix products built concurrently with the chains
"""

import sys, time

sys.path.insert(0, "/opt/trn_rl_repo")

from contextlib import ExitStack

import numpy as np

import concourse.bass as bass
import concourse.tile as tile
from concourse import bacc, mybir



F32 = mybir.dt.float32
BF16 = mybir.dt.bfloat16
FP8 = mybir.dt.float8e4
I32 = mybir.dt.int32

V, D, H, T = 50000, 300, 256, 34
DP = 384  # D padded: 300 data + 1 bias/ones row + zeros
B_LOC = 8
G4 = 4 * H  # 1024
AF = mybir.ActivationFunctionType
ALU = mybir.AluOpType


def build_kernel(S: int) -> bass.Bass:
    NTOK = S * B_LOC
    NTILE = NTOK // 128
    assert NTOK % 128 == 0
    TC = min(512, NTOK)  # token chunk for big GEMMs
    NCHUNK = NTOK // TC
    SH = S // 2

    nc = bacc.Bacc("TRN2", target_bir_lowering=False, debug=False)

    emb = nc.dram_tensor("emb", [V, D], F32, kind="ExternalInput")
    tok_ids = nc.dram_tensor("tok_ids", [128, NTILE], I32, kind="ExternalInput")
    wihT = {
        d: nc.dram_tensor(f"wihT_{d}", [DP, G4], BF16, kind="ExternalInput")
        for d in "fb"
    }
    whhT = {
        d: nc.dram_tensor(f"whhT_{d}", [H, G4], FP8, kind="ExternalInput")
        for d in "fb"
    }
    woutT = nc.dram_tensor("woutT", [2 * H, T], BF16, kind="ExternalInput")
    oh = nc.dram_tensor("oh", [T, NTOK], F32, kind="ExternalInput")
    expT = nc.dram_tensor("expT", [T, T], F32, kind="ExternalInput")
    expTT = nc.dram_tensor("expTT", [T, T], F32, kind="ExternalInput")
    # [T, 1] column vectors
    expStart = nc.dram_tensor("expStart", [T, 1], F32, kind="ExternalInput")
    expEnd = nc.dram_tensor("expEnd", [T, 1], F32, kind="ExternalInput")
    expem_bias = nc.dram_tensor("expem_bias", [T, 1], F32, kind="ExternalInput")
    bout = nc.dram_tensor("bout", [T, 1], F32, kind="ExternalInput")
    ident16 = nc.dram_tensor("ident16", [128, 128], BF16, kind="ExternalInput")
    ident32 = nc.dram_tensor("ident32", [128, 128], F32, kind="ExternalInput")
    out = nc.dram_tensor("out", [1, 2], F32, kind="ExternalOutput")

    with tile.TileContext(nc) as tc, ExitStack() as top:
        cp = top.enter_context(tc.tile_pool(name="const", bufs=1))
        xg_pool = top.enter_context(tc.tile_pool(name="xg", bufs=1))
        hist_pool = top.enter_context(tc.tile_pool(name="hist", bufs=1))

        # ---- constants into SBUF ----
        ids_sb = cp.tile([128, NTILE], I32)
        nc.sync.dma_start(ids_sb[:], tok_ids[:])
        whh_sb = {}
        for d in "fb":
            for k in range(2):
                t_ = cp.tile([128, G4], FP8, tag=f"whh_{d}{k}")
                nc.sync.dma_start(t_[:], whhT[d][128 * k : 128 * (k + 1), :])
                whh_sb[d, k] = t_
        wout_sb = []
        for q in range(4):
            t_ = cp.tile([128, T], BF16, tag=f"wout{q}")
            nc.sync.dma_start(t_[:], woutT[128 * q : 128 * (q + 1), :])
            wout_sb.append(t_)
        i16_sb = cp.tile([128, 128], BF16)
        nc.sync.dma_start(i16_sb[:], ident16[:])
        i32_sb = cp.tile([128, 128], F32)
        nc.sync.dma_start(i32_sb[:], ident32[:])
        expT_sb = cp.tile([T, T], F32, tag="expT")
        nc.sync.dma_start(expT_sb[:], expT[:])
        expTT_sb = cp.tile([T, T], F32, tag="expTT")
        nc.sync.dma_start(expTT_sb[:], expTT[:])
        vec_sb = {}
        for name, dram in (
            ("expStart", expStart),
            ("expEnd", expEnd),
            ("expem_bias", expem_bias),
            ("bout", bout),
        ):
            t_ = cp.tile([T, 1], F32, tag=name)
            nc.sync.dma_start(t_[:], dram[:])
            vec_sb[name] = t_
        ones34 = cp.tile([T, 1], F32, tag="ones34")
        nc.vector.memset(ones34[:], 1.0)
        oh_sb = cp.tile([T, NTOK], F32, tag="oh")
        nc.sync.dma_start(oh_sb[:], oh[:])

        # persistent big tensors
        xg_sb = {d: xg_pool.tile([128, S * 64], BF16, tag=f"xg_{d}", name=f"xg_{d}") for d in "fb"}
        hist_sb = {
            d: hist_pool.tile([128, 2 * NTOK], BF16, tag=f"hist_{d}", name=f"hist_{d}") for d in "fb"
        }
        xg_v = {d: xg_sb[d][:].rearrange("p (s m b) -> p s m b", m=8, b=B_LOC)
                for d in "fb"}

        # ---- pools used by gather/transpose/GEMM (open through recurrence) ----
        mid = ExitStack()
        gp = mid.enter_context(tc.tile_pool(name="gather", bufs=1))
        xtp = mid.enter_context(tc.tile_pool(name="xT", bufs=1))
        wip = mid.enter_context(tc.tile_pool(name="wih", bufs=1))
        ptp = mid.enter_context(tc.tile_pool(name="psum_t", bufs=2, space="PSUM"))
        pxp = mid.enter_context(tc.tile_pool(name="psum_x", bufs=2, space="PSUM"))

        wih_sb = {}
        for d in "fb":
            for k in range(3):
                t_ = wip.tile([128, G4], BF16, tag=f"wih_{d}{k}")
                nc.sync.dma_start(t_[:], wihT[d][128 * k : 128 * (k + 1), :])
                wih_sb[d, k] = t_

        xT = [xtp.tile([128, NTOK], BF16, tag=f"xT{k}", name=f"xT{k}") for k in range(3)]
        # bias/ones row lives at d=320 -> xT[2] partition 64 (32-aligned)
        for p0 in (32, 64, 96):
            nc.vector.memset(xT[2][p0 : p0 + 32, :], 0.0)
        nc.vector.memset(xT[2][64:65, :], 1.0)

        # gather all token-embedding tiles up front (DMA engines, off PE);
        # boundary tiles (chunks f:0 and b:3) first -- they gate the
        # recurrence start.
        GATHER_ORDER = [0, 1, 2, 3, 12, 13, 14, 15, 4, 5, 6, 7, 8, 9, 10, 11]
        x_sb = [None] * NTILE
        for i in GATHER_ORDER:
            t_ = gp.tile([128, D], F32, tag=f"x{i}")
            nc.gpsimd.indirect_dma_start(
                out=t_[:, 0:D],
                out_offset=None,
                in_=emb[:],
                in_offset=bass.IndirectOffsetOnAxis(
                    ap=ids_sb[:, i : i + 1], axis=0
                ),
            )
            x_sb[i] = t_

        # GPSIMD cannot touch PSUM -- alternate PSUM spills between the
        # vector and scalar engines to spread the load.
        _eng = [0]

        def _copy(out, in_):
            _eng[0] ^= 1
            if _eng[0]:
                nc.vector.tensor_copy(out=out, in_=in_)
            else:
                nc.scalar.copy(out=out, in_=in_)

        def transpose_unit(i, k):
            kk = 44 if k == 2 else 128
            pt = ptp.tile([128, 128], F32, tag="pt")
            nc.tensor.transpose(
                out=pt[:kk, :],
                in_=x_sb[i][:, 128 * k : 128 * k + kk],
                identity=i32_sb[:],
            )
            _copy(xT[k][:kk, 128 * i : 128 * (i + 1)], pt[:kk, :])

        def gemm_range(d, m, lo, n):
            px = pxp.tile([128, TC], F32, tag="px", name="px")
            for k in range(3):
                nc.tensor.matmul(
                    px[:, 0:n],
                    lhsT=wih_sb[d, k][:, 128 * m : 128 * (m + 1)],
                    rhs=xT[k][:, lo : lo + n],
                    start=(k == 0),
                    stop=(k == 2),
                )
            s_lo, ns = lo // B_LOC, n // B_LOC
            hc = max(ns // 2, 1)
            pxv = px[:].rearrange("p (s b) -> p s b", b=B_LOC)
            _copy(xg_v[d][:, s_lo : s_lo + hc, m, :], pxv[:, 0:hc, :])
            if hc < ns:
                _copy(xg_v[d][:, s_lo + hc : s_lo + ns, m, :], pxv[:, hc:ns, :])

        def gemm_unit(d, j, m):
            gemm_range(d, m, TC * j, TC)

        # up-front: boundary chunks so the recurrence can start
        for i in (0, 1, 2, 3, 12, 13, 14, 15):
            for k in range(3):
                transpose_unit(i, k)
        for m in range(8):
            gemm_unit("f", 0, m)
        for m in range(8):
            gemm_unit("b", 3, m)

        # interleaved work units, emitted inside the recurrence loop
        foreign: dict[int, list] = {}

        def sched(t, fn, *a):
            foreign.setdefault(t, []).append((fn, a))

        # emissions: exp(emit) for the CRF and raw emit for the gold score.
        # Chunks 1/2 have both hf and hb ready at t=192 and run inside the
        # recurrence; chunks 0/3 only complete at the very end.
        expem_sb = cp.tile([T, NTOK], F32, tag="expem")
        emit_sb = cp.tile([T, NTOK], F32, tag="emit")
        rhs4 = [
            hist_sb["f"][:, 0:NTOK],
            hist_sb["f"][:, NTOK : 2 * NTOK],
            hist_sb["b"][:, 0:NTOK],
            hist_sb["b"][:, NTOK : 2 * NTOK],
        ]

        def emission_unit(j, pool):
            pe_full = pool.tile([128, TC], F32, tag="px", name="pe_full")
            pe_ = pe_full[0:T, :]
            for q in range(4):
                nc.tensor.matmul(
                    pe_,
                    lhsT=wout_sb[q][:],
                    rhs=rhs4[q][:, TC * j : TC * (j + 1)],
                    start=(q == 0),
                    stop=(q == 3),
                )
            nc.scalar.activation(
                expem_sb[:, TC * j : TC * (j + 1)], pe_, AF.Exp,
                bias=vec_sb["expem_bias"][:, 0:1],
            )
            nc.scalar.activation(
                emit_sb[:, TC * j : TC * (j + 1)], pe_, AF.Identity,
                bias=vec_sb["bout"][:, 0:1],
            )

        slot = 2  # first recurrence step that takes foreign work
        for i in (4, 5, 6, 7):
            for k in range(3):
                sched(slot, transpose_unit, i, k); slot += 1
        for m in range(8):
            sched(slot, gemm_unit, "f", 1, m); slot += 1
        for i in (8, 9, 10, 11):
            for k in range(3):
                sched(slot, transpose_unit, i, k); slot += 1
        for m in range(8):
            sched(slot, gemm_unit, "b", 2, m); slot += 1
        assert slot < 62, slot
        slot = 66
        for m in range(8):
            sched(slot, gemm_unit, "f", 2, m); slot += 2
        for m in range(8):
            sched(slot, gemm_unit, "b", 1, m); slot += 2
        assert slot < 126, slot
        slot = 130
        for m in range(8):
            sched(slot, gemm_unit, "f", 3, m); slot += 2
        for m in range(8):
            sched(slot, gemm_unit, "b", 0, m); slot += 2
        assert slot < 190, slot
        sched(194, emission_unit, 1, pxp)
        sched(197, emission_unit, 2, pxp)

        # ---- recurrence ----
        with ExitStack() as ph:
            pgp = {
                d: ph.enter_context(
                    tc.tile_pool(name=f"psum_g{d}", bufs=2, space="PSUM")
                )
                for d in "fb"
            }
            sp = ph.enter_context(tc.tile_pool(name="gates", bufs=4))
            cpool = ph.enter_context(tc.tile_pool(name="cstate", bufs=2))
            hist_v = {
                d: hist_sb[d][:].rearrange("p (k s b) -> p k s b", k=2, b=B_LOC)
                for d in "fb"
            }
            c_prev = {d: None for d in "fb"}
            # phase-interleaved emission across the f/b chains: the ACT and
            # DVE queues are strict FIFO, so a per-direction emission order
            # head-of-line-blocks one chain behind the other's tail ops.
            for t in range(S):
                dirs = [("f", t, t - 1), ("b", S - 1 - t, S - t)]
                if t % 2:  # balance chain latencies: alternate PE group order
                    dirs.reverse()
                pg, sg, thg, t2, t3, cn, thc = {}, {}, {}, {}, {}, {}, {}
                for d, s, s_prev in dirs:
                    pg[d] = pgp[d].tile([128, 64], F32, tag=f"pg{d}", name=f"pg_{d}")
                    nc.tensor.matmul(
                        pg[d][:],
                        lhsT=i16_sb[:],
                        rhs=xg_sb[d][:, 64 * s : 64 * (s + 1)],
                        start=True,
                        stop=(t == 0),
                        skip_group_check=True,
                    )
                    if t > 0:
                        for m in range(8):
                            for k in range(2):
                                nc.tensor.matmul(
                                    pg[d][:, 8 * m : 8 * (m + 1)],
                                    lhsT=whh_sb[d, k][:, 128 * m : 128 * (m + 1)],
                                    rhs=hist_v[d][:, k, s_prev, :],
                                    start=False,
                                    stop=(k == 1 and m == 7),
                                    skip_group_check=True,
                                )
                for d, s, s_prev in dirs:
                    sg[d] = sp.tile([128, 64], F32, tag=f"sig{d}", name=f"sg_{d}")
                    nc.scalar.activation(sg[d][:], pg[d][:], AF.Sigmoid)
                # DVE ops in expected execution-time order: the queue is
                # strict FIFO, so the lead chain's tail must not sit behind
                # the lag chain's head ops.
                for d, s, s_prev in dirs:
                    thg[d] = sp.tile([128, 16], F32, tag=f"tg{d}", name=f"thg_{d}")
                    t2[d] = sp.tile([128, 16], F32, tag=f"t2{d}", name=f"t2_{d}")
                    t3[d] = sp.tile([128, 16], F32, tag=f"t3{d}", name=f"t3_{d}")
                    cn[d] = cpool.tile([128, 16], F32, tag=f"c{d}", name=f"cn_{d}")
                    # tanh(g) = 2*sigmoid(2g) - 1
                    nc.vector.tensor_scalar(
                        out=thg[d][:], in0=sg[d][:, 48:64], scalar1=2.0,
                        scalar2=-1.0, op0=ALU.mult, op1=ALU.add,
                    )
                    if t == 0:
                        nc.vector.tensor_tensor(
                            out=cn[d][:], in0=sg[d][:, 0:16], in1=thg[d][:],
                            op=ALU.mult,
                        )
                    else:
                        nc.vector.tensor_tensor(
                            out=t3[d][:], in0=sg[d][:, 16:32], in1=c_prev[d][:],
                            op=ALU.mult,
                        )
                        nc.vector.tensor_tensor(
                            out=t2[d][:], in0=sg[d][:, 0:16], in1=thg[d][:],
                            op=ALU.mult,
                        )
                        nc.vector.tensor_add(out=cn[d][:], in0=t2[d][:], in1=t3[d][:])
                for d, s, s_prev in dirs:
                    c_prev[d] = cn[d]
                    thc[d] = sp.tile([128, 16], F32, tag=f"thc{d}", name=f"thc_{d}")
                    nc.scalar.activation(thc[d][:], cn[d][:], AF.Tanh)
                for d, s, s_prev in dirs:
                    nc.vector.tensor_tensor(
                        out=hist_v[d][:, :, s, :],
                        in0=sg[d][:, 32:48].rearrange("p (k b) -> p k b", b=B_LOC),
                        in1=thc[d][:].rearrange("p (k b) -> p k b", b=B_LOC),
                        op=ALU.mult,
                    )
                for fn, a in foreign.get(t, ()):
                    fn(*a)

        mid.close()  # free gather/xT/wih SBUF + transpose/GEMM PSUM banks

        # ---- remaining emissions + gold emission score ----
        acc_em = cp.tile([T, 1], F32, tag="acc_em")
        ps_out = top.enter_context(tc.tile_pool(name="psum_o", bufs=1, space="PSUM"))
        po_num = ps_out.tile([1, 1], F32, tag="po_num")
        po_den = ps_out.tile([1, B_LOC], F32, tag="po_den")
        with ExitStack() as ph:
            pep = ph.enter_context(tc.tile_pool(name="psum_e", bufs=2, space="PSUM"))
            ep = ph.enter_context(tc.tile_pool(name="emitp", bufs=1))
            prod_sb = ep.tile([T, NTOK], F32, tag="prod")
            for j in (0, 3):
                emission_unit(j, pep)
            nc.vector.scalar_tensor_tensor(
                out=prod_sb[:],
                in0=emit_sb[:],
                scalar=0.0,
                in1=oh_sb[:],
                op0=ALU.add,
                op1=ALU.mult,
                accum_out=acc_em[:],
            )
            nc.tensor.matmul(
                po_num[:], lhsT=ones34[:], rhs=acc_em[:], start=True, stop=True
            )

        # ---- CRF exp-domain chains, 4-way segmented ----
        # Vector chains cover s in [0, VLEN) fwd and (S-VLEN, S-1] bwd; the
        # middle is bridged by two per-sequence TxT transfer-matrix products
        # G1 (s in [VLEN, VLEN+ML)) and G2 (s in [VLEN+ML, S-VLEN)), built in
        # parallel with the vector chains, then applied transposed:
        # den = Ea_{V-1} . (G1^T G2^T E Eb_{S-V}).
        VLEN = 80
        ML = (S - 2 * VLEN) // 2
        assert S == 2 * VLEN + 2 * ML
        with ExitStack() as ph:
            crf = ph.enter_context(tc.tile_pool(name="crf", bufs=3))
            pcp = ph.enter_context(tc.tile_pool(name="psum_c", bufs=2, space="PSUM"))
            pmp = {
                g: ph.enter_context(
                    tc.tile_pool(name=f"psum_m{g}", bufs=2, space="PSUM"))
                for g in (1, 2)
            }
            gmp = ph.enter_context(tc.tile_pool(name="gmats", bufs=2))

            em = lambda s: expem_sb[:, B_LOC * s : B_LOC * (s + 1)]
            emv = expem_sb[:].rearrange("p (s b) -> p s b", b=B_LOC)
            # paired chains: cols 0:8 = Ea (fwd), cols 8:16 = Eb (bwd)
            eab = crf.tile([T, 2 * B_LOC], F32, tag="eab")
            nc.vector.tensor_scalar_mul(eab[:, 0:B_LOC], em(0), vec_sb["expStart"][:, 0:1])
            nc.vector.tensor_scalar_mul(eab[:, B_LOC:], em(S - 1), vec_sb["expEnd"][:, 0:1])

            # bf16 E for the G-chain matmuls (fp32 lhsT can't pair w/ bf16 rhs)
            expT16 = crf.tile([T, T], BF16, tag="expT16")
            nc.vector.tensor_copy(out=expT16[:], in_=expT_sb[:])
            # G init: diag(em_s0) E^T == row-scaled expTT (bf16, per seq)
            G = {}
            for g, s0 in ((1, VLEN), (2, VLEN + ML)):
                Gt = gmp.tile([T, B_LOC * T], BF16, tag=f"G{g}", name=f"G{g}")
                for b in range(B_LOC):
                    nc.vector.tensor_scalar_mul(
                        Gt[:, T * b : T * (b + 1)], expTT_sb[:],
                        emv[:, s0, b : b + 1])
                G[g] = Gt

            for r in range(1, VLEN):
                # fwd: Ea_r = em(r) * expT.T @ Ea_{r-1}
                # bwd: Eb_{S-1-r} = em(S-1-r) * (Eb chain);  em pair strided AP
                pcab = pcp.tile([T, 2 * B_LOC], F32, tag="pcab")
                nc.tensor.matmul(pcab[:, 0:B_LOC], lhsT=expT_sb[:], rhs=eab[:, 0:B_LOC],
                                 start=True, stop=True)
                nc.tensor.matmul(pcab[:, B_LOC:], lhsT=expTT_sb[:], rhs=eab[:, B_LOC:],
                                 start=True, stop=True)
                eab = crf.tile([T, 2 * B_LOC], F32, tag="eab")
                em_pair = emv[:, r : S - r : S - 1 - 2 * r, :]
                nc.vector.tensor_tensor(
                    out=eab[:].rearrange("p (c b) -> p c b", b=B_LOC),
                    in0=pcab[:].rearrange("p (c b) -> p c b", b=B_LOC),
                    in1=em_pair,
                    op=ALU.mult,
                )
                if r < ML:
                    for g, s0 in ((1, VLEN), (2, VLEN + ML)):
                        pm = pmp[g].tile([T, B_LOC * T], F32, tag=f"pm{g}",
                                         name=f"pm_{g}")
                        nc.tensor.matmul(pm[:], lhsT=expT16[:], rhs=G[g][:],
                                         start=True, stop=True)
                        Gn = gmp.tile([T, B_LOC * T], BF16, tag=f"G{g}",
                                      name=f"Gn_{g}")
                        nc.vector.tensor_tensor(
                            out=Gn[:].rearrange("p (b i) -> p b i", b=B_LOC),
                            in0=pm[:].rearrange("p (b i) -> p b i", b=B_LOC),
                            in1=emv[:, s0 + r, :].to_broadcast([T, B_LOC, T]),
                            op=ALU.mult,
                        )
                        G[g] = Gn

            # tail: v1 = E Eb_{S-V}; v2 = G2^T v1; v3 = G1^T v2;
            # den8 = ones^T (Ea_{V-1} * v3)
            pv1 = pcp.tile([T, B_LOC], F32, tag="pcab")
            nc.tensor.matmul(pv1[:], lhsT=expTT_sb[:], rhs=eab[:, B_LOC:],
                             start=True, stop=True)
            v1 = crf.tile([T, B_LOC], BF16, tag="v1")
            nc.vector.tensor_copy(out=v1[:], in_=pv1[:])
            pv2 = pcp.tile([T, B_LOC], F32, tag="pcab")
            for b in range(B_LOC):
                nc.tensor.matmul(pv2[:, b : b + 1], lhsT=G[2][:, T * b : T * (b + 1)],
                                 rhs=v1[:, b : b + 1], start=True, stop=True)
            v2 = crf.tile([T, B_LOC], BF16, tag="v1")
            nc.vector.tensor_copy(out=v2[:], in_=pv2[:])
            pv3 = pcp.tile([T, B_LOC], F32, tag="pcab")
            for b in range(B_LOC):
                nc.tensor.matmul(pv3[:, b : b + 1], lhsT=G[1][:, T * b : T * (b + 1)],
                                 rhs=v2[:, b : b + 1], start=True, stop=True)
            z = crf.tile([T, B_LOC], F32, tag="z")
            nc.vector.tensor_tensor(out=z[:], in0=eab[:, 0:B_LOC], in1=pv3[:], op=ALU.mult)
            nc.tensor.matmul(po_den[:], lhsT=ones34[:], rhs=z[:], start=True, stop=True)

            outv = crf.tile([1, 2], F32, tag="outv")
            den8 = crf.tile([1, B_LOC], F32, tag="den8")
            nc.scalar.activation(den8[:], po_den[:], AF.Ln)
            nc.vector.reduce_sum(
                out=outv[:, 1:2], in_=den8[:], axis=mybir.AxisListType.X
            )
            nc.vector.tensor_copy(out=outv[:, 0:1], in_=po_num[:])
            nc.sync.dma_start(out[:], outv[:])

    nc.compile()
    return nc


# ----- host-side preprocessing -----
GATE_PERM = np.concatenate(
    [np.arange(0, 2 * H), np.arange(3 * H, 4 * H), np.arange(2 * H, 3 * H)]
)


def prep_shared(w_ih_f, w_hh_f, b_f, w_ih_b, w_hh_b, b_b, w_out, b_out,
                start_t, end_t, trans):
    """Per-core-replicated tensors, keyed by dram tensor name."""
    import ml_dtypes
    out = {}
    for d, w_ih, b in (("f", w_ih_f, b_f), ("b", w_ih_b, b_b)):
        wp = np.zeros((DP, G4), np.float32)
        wp[:D] = w_ih[GATE_PERM].T.astype(np.float32)
        wp[320] = b[GATE_PERM].astype(np.float32)  # bias row at 32-aligned partition
        wp[:, 3 * H :] *= 2.0  # g-gate pre-scale: tanh(x) = 2*sigmoid(2x) - 1
        out[f"wihT_{d}"] = wp.astype(ml_dtypes.bfloat16)
    for d, w_hh in (("f", w_hh_f), ("b", w_hh_b)):
        whp = w_hh[GATE_PERM].T.astype(np.float32)
        whp[:, 3 * H :] *= 2.0  # g-gate pre-scale for the sigmoid-only gate op
        out[f"whhT_{d}"] = whp.astype(ml_dtypes.float8_e4m3)
    out["woutT"] = w_out.T.astype(ml_dtypes.bfloat16)
    out["expT"] = np.exp(trans).astype(np.float32)
    out["expTT"] = np.exp(trans).T.copy().astype(np.float32)
    out["expStart"] = np.exp(start_t).astype(np.float32).reshape(T, 1)
    out["expEnd"] = np.exp(end_t).astype(np.float32).reshape(T, 1)
    out["expem_bias"] = (b_out - np.log(T)).astype(np.float32).reshape(T, 1)
    out["bout"] = b_out.astype(np.float32).reshape(T, 1)
    out["ident16"] = np.eye(128, dtype=ml_dtypes.bfloat16)
    out["ident32"] = np.eye(128, dtype=np.float32)
    return out


def prep_core(batch_sh, tags_sh, S):
    """Per-core tensors from this core's [B_LOC, S] int shards."""
    ntok = S * B_LOC
    ntile = ntok // 128
    ids_flat = batch_sh.T.reshape(-1).astype(np.int32)  # s-major token order
    tok_ids = ids_flat.reshape(ntile, 128).T.copy()
    oh = np.zeros((T, ntok), np.float32)
    tags_flat = tags_sh.T.reshape(-1)  # [ntok] s-major
    oh[tags_flat, np.arange(ntok)] = 1.0
    return {"tok_ids": tok_ids, "oh": oh}


def num_host(tags, start_t, end_t, trans):
    """Tag-path score pieces that don't involve emissions. tags: [B, S]."""
    return float(
        start_t[tags[:, 0]].sum()
        + trans[tags[:, :-1], tags[:, 1:]].sum()
        + end_t[tags[:, -1]].sum()
    )



# ---------------------------------------------------------------------------
# SPMD runner (the run_bass_kernel_spmd axon path, kept open for re-timing).

S_FULL = 256
N_CORES = 8
LAST_EXEC_NS = None
LAST_IN_MAPS = None

_built = {}


def _get_nc():
    if "nc" not in _built:
        _built["nc"] = build_kernel(S_FULL)
    return _built["nc"]


def _run_spmd_timed(nc, in_maps, n_reps=3):
    """bass2jax.run_bass_via_pjrt equivalent that keeps the jitted executable
    and device-resident inputs so pure-execution time can be measured."""
    global LAST_EXEC_NS
    import jax
    from jax.sharding import Mesh, PartitionSpec, NamedSharding
    from jax.experimental.shard_map import shard_map
    from concourse import bass2jax
    from concourse.bass2jax import _bass_exec_p, partition_id_tensor

    bass2jax.install_neuronx_cc_hook()
    partition_name = nc.partition_id_tensor.name if nc.partition_id_tensor else None

    in_names, out_names, out_avals, zero_outs = [], [], [], []
    for alloc in nc.m.functions[0].allocations:
        if not isinstance(alloc, mybir.MemoryLocationSet):
            continue
        name = alloc.memorylocations[0].name
        if alloc.kind == "ExternalInput":
            if name != partition_name:
                in_names.append(name)
        elif alloc.kind == "ExternalOutput":
            shape = tuple(alloc.tensor_shape)
            dtype = mybir.dt.np(alloc.dtype)
            out_names.append(name)
            out_avals.append(jax.core.ShapedArray(shape, dtype))
            zero_outs.append(np.zeros(shape, dtype))
    n_params = len(in_names)
    n_outs = len(out_avals)
    in_names.extend(out_names)
    if partition_name is not None:
        in_names.append(partition_name)

    donate = tuple(range(n_params, n_params + n_outs))

    def _body(*args):
        operands = list(args)
        if partition_name is not None:
            operands.append(partition_id_tensor())
        return tuple(
            _bass_exec_p.bind(
                *operands,
                out_avals=tuple(out_avals),
                in_names=tuple(in_names),
                out_names=tuple(out_names),
                lowering_input_output_aliases=(),
                sim_require_finite=True,
                sim_require_nnan=True,
                nc=nc,
            )
        )

    devices = jax.devices()[:N_CORES]
    mesh = Mesh(np.asarray(devices), ("core",))
    in_specs = (PartitionSpec("core"),) * (n_params + n_outs)
    out_specs = (PartitionSpec("core"),) * n_outs
    sharded = jax.jit(
        shard_map(_body, mesh=mesh, in_specs=in_specs, out_specs=out_specs,
                  check_rep=False),
        donate_argnums=donate,
        keep_unused=True,
    )
    sh = NamedSharding(mesh, PartitionSpec("core"))
    concat_in = [
        jax.device_put(
            np.concatenate([np.asarray(m[in_names[i]]) for m in in_maps], axis=0), sh
        )
        for i in range(n_params)
    ]

    def zeros():
        return [np.zeros((N_CORES * z.shape[0], *z.shape[1:]), z.dtype)
                for z in zero_outs]

    out_arrs = [np.asarray(a) for a in sharded(*concat_in, *zeros())]
    times = []
    for _ in range(n_reps):
        t0 = time.perf_counter()
        r = sharded(*concat_in, *zeros())
        jax.block_until_ready(r)
        times.append(time.perf_counter() - t0)
    if times:
        LAST_EXEC_NS = int(min(times) * 1e9)
    return [
        {name: out_arrs[i].reshape(N_CORES, *out_avals[i].shape)[c]
         for i, name in enumerate(out_names)}
        for c in range(N_CORES)
    ]


def kernel(batch, tags, seq_lengths, emb, w_ih_f, w_hh_f, b_f,
           w_ih_b, w_hh_b, b_b, w_out, b_out, start_t, end_t, trans):
    global LAST_IN_MAPS
    batch = np.asarray(batch)
    tags = np.asarray(tags)
    emb = np.asarray(emb, np.float32)
    w_out_ = np.asarray(w_out, np.float32)
    b_out_ = np.asarray(b_out, np.float32)
    start_t = np.asarray(start_t, np.float32)
    end_t = np.asarray(end_t, np.float32)
    trans = np.asarray(trans, np.float32)
    S = batch.shape[1]
    assert S == S_FULL and batch.shape[0] == N_CORES * B_LOC

    shared = prep_shared(np.asarray(w_ih_f, np.float32), np.asarray(w_hh_f, np.float32),
                         np.asarray(b_f, np.float32), np.asarray(w_ih_b, np.float32),
                         np.asarray(w_hh_b, np.float32), np.asarray(b_b, np.float32),
                         w_out_, b_out_, start_t, end_t, trans)
    shared["emb"] = emb
    in_maps = []
    for c in range(N_CORES):
        m = dict(shared)
        m.update(prep_core(batch[B_LOC * c : B_LOC * (c + 1)].astype(np.int64),
                           tags[B_LOC * c : B_LOC * (c + 1)].astype(np.int64), S))
        in_maps.append(m)
    LAST_IN_MAPS = in_maps

    nc = _get_nc()
    res = _run_spmd_timed(nc, in_maps)

    num_em_tot = 0.0
    den_raw_tot = 0.0
    for c in range(N_CORES):
        o = np.asarray(res[c]["out"], np.float64).reshape(2)
        num_em_tot += o[0]
        den_raw_tot += o[1]
    den_true_tot = den_raw_tot + N_CORES * B_LOC * S * np.log(T)
    nh = num_host(tags, start_t.astype(np.float64), end_t.astype(np.float64),
                  trans.astype(np.float64))
    llh_tot = nh + num_em_tot - den_true_tot
    loss = -llh_tot / (N_CORES * B_LOC)
    return np.asarray(loss, dtype=np.float32)
